# revision 12
# baseline (speedup 1.0000x reference)
"""Multi-head attention (embed 1024, 16 heads x 64) on 8 TRN2 NeuronCores.

Sharding: tensor-parallel over heads — each core owns 2 heads end-to-end
(qkv projection columns + attention), then an AllToAll redistributes the
per-head attention outputs so each core computes the out-projection for its
1/8 slice of the (batch*seq) rows.

Per-core layout choices (all f32, matmuls in float32r):
  - x is PE-transposed to xT [e, t] so every projection contracts e on
    partitions; projections produce Q/K/V TRANSPOSED [head_dim, t].
  - scores are computed transposed: St[tk, tq] = Kt.T @ Qt, so the softmax
    sum over tk happens on the PE via a ones-column appended to V
    (PV matmul outputs [65, tq]: rows 0..63 = sum exp*V, row 64 = sum exp).
  - normalization: reciprocal of row 64, broadcast across 64 partitions with
    a K=1 matmul, multiplied in on the vector engine.
  - out_proj consumes the AllToAll output directly (head-dim on partitions)
    and the result is PE-transposed back to row-major before the output DMA.
"""

import numpy as np

import concourse.bass as bass
import concourse.tile as tile
from concourse import bacc, mybir
from concourse.bass_utils import run_bass_kernel_spmd
from concourse.masks import make_identity

N_CORES = 8
B, S, D = 2, 2048, 1024
T = B * S              # 4096 flattened tokens
HEADS = 16
DH = 64                # head dim
HPC = HEADS // N_CORES  # heads per core = 2
CW = HPC * DH          # per-core qkv width = 128
SCALE = DH ** -0.5
TC = T // N_CORES      # per-core output row chunk = 512
ET = D // 128          # e partition tiles = 8
F32 = mybir.dt.float32
F32R = mybir.dt.float32r
EXP = mybir.ActivationFunctionType.Exp

_CACHED_NC = None


def build():
    nc = bacc.Bacc(
        "TRN2",
        target_bir_lowering=False,
        debug=False,
        num_devices=N_CORES,
    )
    x_ap = nc.dram_tensor("x", [T, D], F32, kind="ExternalInput").ap()
    wq_ap = nc.dram_tensor("wq", [D, CW], F32, kind="ExternalInput").ap()
    wk_ap = nc.dram_tensor("wk", [D, CW], F32, kind="ExternalInput").ap()
    wv_ap = nc.dram_tensor("wv", [D, CW], F32, kind="ExternalInput").ap()
    bq_ap = nc.dram_tensor("bq", [CW, 1], F32, kind="ExternalInput").ap()
    bk_ap = nc.dram_tensor("bk", [CW, 1], F32, kind="ExternalInput").ap()
    bv_ap = nc.dram_tensor("bv", [CW, 1], F32, kind="ExternalInput").ap()
    wout_ap = nc.dram_tensor("wout", [D, D], F32, kind="ExternalInput").ap()
    bout_ap = nc.dram_tensor("bout", [128, ET], F32, kind="ExternalInput").ap()
    out_ap = nc.dram_tensor("out", [TC, D], F32, kind="ExternalOutput").ap()

    with tile.TileContext(nc) as tc:
        with (
            tc.tile_pool(name="singles", bufs=1) as singles,
            tc.tile_pool(name="xn", bufs=3) as xn_pool,
            tc.tile_pool(name="xt", bufs=1) as xt_pool,
            tc.tile_pool(name="vt", bufs=2) as vt_pool,
            tc.tile_pool(name="exp", bufs=3) as exp_pool,
            tc.tile_pool(name="fo", bufs=2) as fo_pool,
            tc.tile_pool(name="wo", bufs=2) as wo_pool,
            tc.tile_pool(name="small", bufs=2) as small_pool,
            tc.tile_pool(name="mmps", bufs=2, space="PSUM") as mmps,
            tc.tile_pool(name="stps", bufs=2, space="PSUM") as stps,
            tc.tile_pool(name="pvps", bufs=1, space="PSUM") as pvps,
            tc.tile_pool(name="dram", bufs=1, space="DRAM") as dram,
        ):
            # ---- A2A bounce buffers (collectives need internal DRAM) ----
            a2a_in = dram.tile([D, TC], F32)
            a2a_out = dram.tile([D, TC], F32)

            # ---- constants / weights resident in SBUF ----
            ident = singles.tile([128, 128], F32)
            make_identity(nc, ident)
            ones64 = singles.tile([1, DH], F32R)
            nc.vector.memset(ones64.bitcast(F32), 1.0)

            w_sb, b_sb = {}, {}
            for name, wap, bap in (
                ("q", wq_ap, bq_ap), ("k", wk_ap, bk_ap), ("v", wv_ap, bv_ap)
            ):
                w_sb[name] = singles.tile(
                    [128, ET, CW], F32R, tag=f"w{name}", name=f"w{name}_sb"
                )
                nc.sync.dma_start(
                    out=w_sb[name],
                    in_=wap.rearrange("(et p) c -> p et c", p=128).bitcast(F32R)
                )
                b_sb[name] = singles.tile(
                    [CW, 1], F32, tag=f"b{name}", name=f"b{name}_sb"
                )
                nc.sync.dma_start(out=b_sb[name], in_=bap)
            bout_sb = singles.tile([128, ET], F32)
            nc.sync.dma_start(out=bout_sb, in_=bout_ap)

            # persistent activations
            qt = singles.tile([CW, T], F32R, tag="qt")      # [2h*64, t] transposed Q
            kt = singles.tile([CW, T], F32R, tag="kt")
            # V natural per head, 65-wide tk-tiles (col 64 = ones for denom)
            vsb = [
                singles.tile(
                    [128, T // 128, DH + 1], F32R, tag=f"v{h}", name=f"v{h}_sb"
                )
                for h in range(HPC)
            ]
            for h in range(HPC):
                nc.vector.memset(vsb[h][:, :, DH:DH + 1].bitcast(F32), 1.0)

            # ---- stage A: transpose x + qkv projections (per 512-token chunk) ----
            for tch in range(T // 512):
                xt_sb = xt_pool.tile([128, ET, 512], F32R)
                for tt in range(4):
                    xn = xn_pool.tile([128, D], F32)
                    row0 = tch * 512 + tt * 128
                    nc.sync.dma_start(out=xn, in_=x_ap[row0:row0 + 128, :])
                    for et in range(ET):
                        ps = mmps.tile([128, 128], F32, tag="mm")
                        nc.tensor.transpose(
                            ps, xn[:, et * 128:(et + 1) * 128], ident
                        )
                        nc.vector.tensor_copy(
                            xt_sb[:, et, tt * 128:(tt + 1) * 128], ps
                        )
                for name, dest in (("q", qt), ("k", kt), ("v", None)):
                    pp = mmps.tile([CW, 512], F32, tag="mm")
                    for et in range(ET):
                        nc.tensor.matmul(
                            pp,
                            (w_sb[name][:, et, :]),
                            (xt_sb[:, et, :]),
                            start=(et == 0),
                            stop=(et == ET - 1),
                        )
                    if dest is not None:
                        nc.vector.tensor_scalar_add(
                            dest[:, tch * 512:(tch + 1) * 512], pp, b_sb[name]
                        )
                    else:
                        vt_tmp = vt_pool.tile([CW, 512], F32)
                        nc.vector.tensor_scalar_add(vt_tmp, pp, b_sb[name])
                        for tt in range(4):
                            ps2 = mmps.tile([128, 128], F32, tag="mm")
                            nc.tensor.transpose(
                                ps2, vt_tmp[:, tt * 128:(tt + 1) * 128], ident
                            )
                            ttg = tch * 4 + tt
                            for h in range(HPC):
                                nc.vector.tensor_copy(
                                    vsb[h][:, ttg, 0:DH],
                                    ps2[:, h * DH:(h + 1) * DH],
                                )

                # ---- stage B: attention for batch b once its chunks are done ----
                if tch == 3 or tch == 7:
                    b = tch // 4
                    for h in range(HPC):
                        po = h * DH
                        for tqh in range(2):  # 1024-wide tq slabs
                            tq0 = b * S + tqh * 1024
                            pv = pvps.tile([DH + 1, 1024], F32)
                            for tkt in range(16):
                                st = stps.tile([128, 1024], F32, tag="st")
                                k0 = b * S + tkt * 128
                                for nh in range(2):
                                    nc.tensor.matmul(
                                        st[:, nh * 512:(nh + 1) * 512],
                                        (kt[po:po + DH, k0:k0 + 128]),
                                        (qt[po:po + DH,
                                              tq0 + nh * 512:tq0 + (nh + 1) * 512]),
                                    )
                                ex = exp_pool.tile([128, 1024], F32R)
                                nc.scalar.activation(ex, st, EXP)
                                for nh in range(2):
                                    nc.tensor.matmul(
                                        pv[:, nh * 512:(nh + 1) * 512],
                                        (vsb[h][:, b * 16 + tkt, :]),
                                        (ex[:, nh * 512:(nh + 1) * 512]),
                                        start=(tkt == 0),
                                        stop=(tkt == 15),
                                    )
                            recip = small_pool.tile([1, 1024], F32R)
                            with nc.allow_low_precision(
                                reason="f32r keeps 4-byte storage; feeds bcast matmul"
                            ):
                                nc.vector.reciprocal(recip, pv[DH:DH + 1, :])
                            bc = stps.tile([DH, 1024], F32, tag="st")
                            for nh in range(2):
                                nc.tensor.matmul(
                                    bc[:, nh * 512:(nh + 1) * 512],
                                    (ones64),
                                    (recip[:, nh * 512:(nh + 1) * 512]),
                                )
                            pvc = fo_pool.tile([DH, 1024], F32, tag="pvc", name="pvc")
                            nc.vector.tensor_copy(pvc, pv[0:DH, :])
                            fo = fo_pool.tile([DH, 1024], F32)
                            nc.vector.tensor_mul(fo, pvc, bc)
                            # scatter into a2a input: shard j rows = j*128..,
                            # this head occupies partitions po..po+64 of shard
                            t0 = b * S + tqh * 1024  # global t of fo col 0
                            a2a_view = a2a_in[:, :].rearrange(
                                "(j p) t -> p j t", p=128
                            )
                            j0 = t0 // TC  # first shard index (chunks of 512)
                            nc.sync.dma_start(
                                out=a2a_view[po:po + DH, j0:j0 + 2, :],
                                in_=fo.rearrange("p (j t) -> p j t", j=2),
                            )

            # ---- stage C: AllToAll of head-features ----
            nc.gpsimd.collective_compute(
                "AllToAll",
                mybir.AluOpType.bypass,
                replica_groups=[list(range(N_CORES))],
                ins=[a2a_in[:, :].opt()],
                outs=[a2a_out[:, :].opt()],
            )
            g_sb = singles.tile([128, ET, TC], F32R, tag="g")
            nc.sync.dma_start(
                out=g_sb,
                in_=a2a_out[:, :].rearrange("(j p) t -> p j t", p=128).bitcast(F32R)
            )

            # ---- stage D: out projection on this core's 512-token chunk ----
            for ot in range(ET):
                wo_sb = wo_pool.tile([128, ET, 128], F32R)
                nc.sync.dma_start(
                    out=wo_sb,
                    in_=wout_ap[:, ot * 128:(ot + 1) * 128].rearrange(
                        "(ht p) o -> p ht o", p=128
                    ).bitcast(F32R),
                )
                pp = mmps.tile([128, TC], F32, tag="mm")
                for ht in range(ET):
                    nc.tensor.matmul(
                        pp,
                        (wo_sb[:, ht, :]),
                        (g_sb[:, ht, :]),
                        start=(ht == 0),
                        stop=(ht == ET - 1),
                    )
                ob = vt_pool.tile([128, TC], F32)
                nc.vector.tensor_scalar_add(ob, pp, bout_sb[:, ot:ot + 1])
                for tt in range(4):
                    ps = mmps.tile([128, 128], F32, tag="mm")
                    nc.tensor.transpose(
                        ps, ob[:, tt * 128:(tt + 1) * 128], ident
                    )
                    on = small_pool.tile([128, 128], F32, tag="on", name="on")
                    nc.vector.tensor_copy(on, ps)
                    nc.sync.dma_start(
                        out=out_ap[tt * 128:(tt + 1) * 128,
                                   ot * 128:(ot + 1) * 128],
                        in_=on,
                    )
    nc.compile()
    return nc


def shard_inputs(x, w_qkv, b_qkv, w_out, b_out):
    """Split full inputs into the 8 per-core input maps."""
    x2d = np.ascontiguousarray(x.reshape(T, D).astype(np.float32))
    w_qkv = np.asarray(w_qkv, dtype=np.float32)
    b_qkv = np.asarray(b_qkv, dtype=np.float32)
    w_out = np.ascontiguousarray(np.asarray(w_out, dtype=np.float32))
    b_out = np.asarray(b_out, dtype=np.float32)
    bout_r = np.ascontiguousarray(b_out.reshape(ET, 128).T)  # [p, ot]
    in_maps = []
    for i in range(N_CORES):
        c0 = i * CW
        wq = np.ascontiguousarray(w_qkv[:, c0:c0 + CW]) * SCALE
        wk = np.ascontiguousarray(w_qkv[:, D + c0:D + c0 + CW])
        wv = np.ascontiguousarray(w_qkv[:, 2 * D + c0:2 * D + c0 + CW])
        bq = (b_qkv[c0:c0 + CW] * SCALE).reshape(CW, 1)
        bk = b_qkv[D + c0:D + c0 + CW].reshape(CW, 1)
        bv = b_qkv[2 * D + c0:2 * D + c0 + CW].reshape(CW, 1)
        in_maps.append({
            "x": x2d,
            "wq": wq, "wk": wk, "wv": wv,
            "bq": np.ascontiguousarray(bq),
            "bk": np.ascontiguousarray(bk),
            "bv": np.ascontiguousarray(bv),
            "wout": w_out,
            "bout": bout_r,
        })
    return in_maps


def get_nc():
    global _CACHED_NC
    if _CACHED_NC is None:
        _CACHED_NC = build()
    return _CACHED_NC


def run(in_maps, trace=False, **kw):
    nc = get_nc()
    return run_bass_kernel_spmd(
        nc, in_maps, core_ids=list(range(N_CORES)), trace=trace, **kw
    )


def kernel(x, w_qkv, b_qkv, w_out, b_out):
    in_maps = shard_inputs(x, w_qkv, b_qkv, w_out, b_out)
    res = run(in_maps, trace=False)
    full = np.concatenate([r["out"] for r in res.results], axis=0)
    return full.reshape(B, S, D)
